# revision 1
# baseline (speedup 1.0000x reference)
"""Trainium2 Bass kernel for nn_CombinedRepeatCausalLinear (PE version).

Math: out[r, t] = sum_{s<=t} x[r, s] * (w0[s]*dv0^(t-s) + w1[t]*dv1^(t-s)) + bias[t]

Chunked linear-attention formulation (chunk L=126 along S):
  - Diagonal blocks D_c[s_l, t_l] (upper-triangular, [128,128] with 2 extra
    "reduction" columns producing decay-weighted chunk sums S0_c, S1_c in
    psum rows 126/127).
  - Cross-chunk contribution is rank-2 per source chunk:
      out[t] += sum_{c'<c(t)} dv0^(t-e_c')*S1_c'[r] + w1[t]*dv1^(t-e_c')*S0_c'[r]
    implemented as a second matmul per chunk against a host-built matrix.

All matmuls are K=128 (host matrices zero-padded) so the PE array stays
fully active and the HAM clock-gate holds the warm 2.4 GHz state; fp32
throughout (HI/LO 2-pass, exact fp32 products). The output is computed
transposed (t on partitions) so the host-built matrices are always the
stationary operand; the host ships x pre-transposed per shard and
transposes the gathered result back.

Data-parallel across 8 NeuronCores on the fused B*E axis.
"""

import sys

if "/opt/trn_rl_repo" not in sys.path:
    sys.path.insert(0, "/opt/trn_rl_repo")

import numpy as np

import concourse.mybir as mybir
from concourse import bacc
from concourse.bass_utils import run_bass_kernel_spmd
from concourse.mybir import AluOpType
from concourse.tile import TileContext

_P = 128
_B, _E, _S = 4, 2048, 2048
_NCORES = 8
_R = (_B * _E) // _NCORES  # 1024 rows (r) per core
_L = 126  # chunk length along S
_NCH = (_S + _L - 1) // _L  # 17 chunks (last has 32)
_HALF = 512  # r per matmul (one PSUM bank, fp32)
_NH = _R // _HALF  # 2 halves

_F32 = mybir.dt.float32


def _chunk_len(c):
    return min(_L, _S - c * _L)


def _build_host_mats(w0, w1, dv0, dv1, bias, with_bias):
    """Build D [128, NCH*128] and M [128, NCH*128] in float64, cast f32."""
    w0 = w0.astype(np.float64)
    w1 = w1.astype(np.float64)
    D = np.zeros((_NCH * _P, _P), dtype=np.float64)
    for c in range(_NCH):
        Lc = _chunk_len(c)
        base = c * _L
        sl = np.arange(Lc)
        tl = np.arange(Lc)
        diff = tl[None, :] - sl[:, None]
        mask = diff >= 0
        blk = np.where(
            mask,
            w0[base + sl][:, None] * (dv0 ** np.maximum(diff, 0))
            + w1[base + tl][None, :] * (dv1 ** np.maximum(diff, 0)),
            0.0,
        )
        Db = D[c * _P : (c + 1) * _P]
        Db[:Lc, :Lc] = blk
        # reduction columns: col 126 -> S0_c (dv1-weighted sum),
        #                    col 127 -> S1_c (w0*dv0-weighted sum)
        Db[:Lc, 126] = dv1 ** (Lc - 1 - sl)
        Db[:Lc, 127] = w0[base + sl] * dv0 ** (Lc - 1 - sl)

    off = 1 if with_bias else 0
    # M padded to 128 contraction rows (rows >= off+2*NCH are zero)
    M = np.zeros((_P, _NCH * _P), dtype=np.float64)
    for c in range(_NCH):
        Lc = _chunk_len(c)
        t = c * _L + np.arange(Lc)
        if with_bias:
            M[0, c * _P : c * _P + Lc] = bias.astype(np.float64)[t]
        for cp in range(c):
            e_cp = cp * _L + _chunk_len(cp) - 1
            M[off + 2 * cp, c * _P : c * _P + Lc] = w1[t] * (dv1 ** (t - e_cp))
            M[off + 2 * cp + 1, c * _P : c * _P + Lc] = dv0 ** (t - e_cp)
    return D.astype(np.float32), M.astype(np.float32)


def _build(with_bias):
    off = 1 if with_bias else 0
    nc = bacc.Bacc(
        "TRN2",
        target_bir_lowering=False,
        debug=False,
        enable_asserts=False,
        num_devices=_NCORES,
    )
    xt = nc.dram_tensor("xt", [_S, _R], _F32, kind="ExternalInput").ap()
    Dd = nc.dram_tensor("Dd", [_NCH * _P, _P], _F32, kind="ExternalInput").ap()
    Md = nc.dram_tensor("Md", [_P, _NCH * _P], _F32, kind="ExternalInput").ap()
    outT = nc.dram_tensor("outT", [_S, _R], _F32, kind="ExternalOutput").ap()

    with TileContext(nc) as tc:
        with (
            tc.tile_pool(name="consts", bufs=1) as cpool,
            tc.tile_pool(name="xin", bufs=8) as xpool,
            tc.tile_pool(name="dg", bufs=1) as dgpool,
            tc.tile_pool(name="ot", bufs=4) as otpool,
            tc.tile_pool(name="pd", bufs=5, space="PSUM") as pdpool,
            tc.tile_pool(name="po", bufs=3, space="PSUM") as popool,
        ):
            sall = cpool.tile([_P, _R], _F32)
            nc.gpsimd.memset(sall[:], 0.0)
            if with_bias:
                nc.gpsimd.memset(sall[0:1, :], 1.0)
            # dedicated last-chunk x tile, zero-filled up front so the
            # memset is off the phase-1 critical path (K=128 contraction
            # reads the zero tail rows)
            xlast = cpool.tile([_P, _R], _F32)
            nc.gpsimd.memset(xlast[:], 0.0)

            # ---- Phase 1: diagonal blocks + chunk reductions ----
            Dt = None
            Mt = None
            dg_tiles = []
            for c in range(_NCH):
                Lc = _chunk_len(c)
                rows = min(_P, _S - c * _L)  # 128, except last chunk: 32
                if rows < _P:
                    xtile = xlast
                else:
                    xtile = xpool.tile([_P, _R], _F32, tag="x", name="x")
                nc.sync.dma_start(xtile[:rows, :], xt[c * _L : c * _L + rows, :])
                dtile = xpool.tile([_P, _P], _F32, tag="d", name="d")
                nc.scalar.dma_start(dtile[:], Dd[c * _P : (c + 1) * _P, :])
                if c == 8:
                    # M is only needed for phase 2; load it mid-phase
                    Mt = cpool.tile([_P, _NCH * _P], _F32)
                    nc.scalar.dma_start(Mt[:], Md[:])
                dg = dgpool.tile([_P, _R], _F32, tag=f"dg{c}", name="dg")
                for h in range(_NH):
                    pd = pdpool.tile([_P, _HALF], _F32, tag="pd", name="pd")
                    nc.tensor.matmul(
                        pd[:],
                        dtile[:],
                        xtile[:, h * _HALF : (h + 1) * _HALF],
                        start=True,
                        stop=True,
                    )
                    nc.vector.tensor_copy(dg[:, h * _HALF : (h + 1) * _HALF], pd[:])
                    # move the chunk-sum rows into Sall partitions (2c, 2c+1)
                    nc.gpsimd.dma_start(
                        sall[off + 2 * c : off + 2 * c + 2, h * _HALF : (h + 1) * _HALF],
                        dg[126:128, h * _HALF : (h + 1) * _HALF],
                    )
                dg_tiles.append(dg)

            # ---- Phase 2: cross-chunk offsets + combine + store ----
            for c in range(_NCH):
                Lc = _chunk_len(c)
                dg = dg_tiles[c]
                if c == 0 and not with_bias:
                    nc.sync.dma_start(outT[0:_L, :], dg[:_L, :])
                    continue
                ot = otpool.tile([_P, _R], _F32, tag="ot", name="ot")
                for h in range(_NH):
                    po = popool.tile([_P, _HALF], _F32, tag="po", name="po")
                    nc.tensor.matmul(
                        po[:],
                        Mt[:, c * _P : (c + 1) * _P],
                        sall[:, h * _HALF : (h + 1) * _HALF],
                        start=True,
                        stop=True,
                    )
                    nc.vector.tensor_tensor(
                        ot[:, h * _HALF : (h + 1) * _HALF],
                        dg[:, h * _HALF : (h + 1) * _HALF],
                        po[:],
                        AluOpType.add,
                    )
                eng = nc.sync if c % 2 == 0 else nc.scalar
                eng.dma_start(outT[c * _L : c * _L + Lc, :], ot[:Lc, :])
    nc.compile()
    return nc


def _run(x, weight, bias, decay_value, trace=False):
    x = np.asarray(x, dtype=np.float32)
    w = np.asarray(weight, dtype=np.float32)
    b = np.asarray(bias, dtype=np.float32)
    dv = np.asarray(decay_value, dtype=np.float32)
    dv0 = float(np.clip(dv[0, 0], 0.9, 1.0))
    dv1 = float(np.clip(dv[1, 0], 0.9, 1.0))
    with_bias = bool(np.any(b))

    D, M = _build_host_mats(w[0], w[1], dv0, dv1, b, with_bias)
    nc = _build(with_bias)

    xf = x.reshape(_B * _E, _S)
    xT = np.ascontiguousarray(xf.T)  # [S, B*E]
    in_maps = []
    for c in range(_NCORES):
        in_maps.append(
            {
                "xt": np.ascontiguousarray(xT[:, c * _R : (c + 1) * _R]),
                "Dd": D,
                "Md": M,
            }
        )

    res = run_bass_kernel_spmd(nc, in_maps, core_ids=list(range(_NCORES)), trace=trace)
    outT = np.concatenate(
        [res.results[c]["outT"] for c in range(_NCORES)], axis=1
    )  # [S, B*E]
    full = np.ascontiguousarray(outT.T).reshape(_B, _E, _S)
    return full, res


def kernel(x, weight, bias, decay_value):
    full, _ = _run(x, weight, bias, decay_value, trace=False)
    return full



# revision 7
# speedup vs baseline: 1.7547x; 1.7547x over previous
"""Trainium2 Bass kernel for nn_CombinedRepeatCausalLinear (bf16 version).

Math: out[r, t] = sum_{s<=t} x[r, s] * (w0[s]*dv0^(t-s) + w1[t]*dv1^(t-s)) + bias[t]

Computed transposed (t on partitions), data-parallel over the fused B*E
axis across 8 NeuronCores (r = 1024 rows per core). Everything on-device
is bf16 (inputs/outputs/weight matrices); the 2e-2 rel-err budget has
orders of magnitude of slack (measured denom absmax ~539, fp32 baseline
abs err ~3e-4, bf16 adds ~1 abs err worst case).

Chunked linear-attention formulation, chunk L=128 along S (16 chunks):

  Phase 1 (sums): for each chunk c, a matmul with stationary G_c
    [128, 33] (two nonzero columns: decay-weighted ones and w0 weights)
    accumulates the per-chunk reductions S0_c, S1_c into a standing
    PSUM bank (rows 1+2c, 2+2c), one bank per 512-column r-half.
    One aligned PSUM->SBUF copy produces `sall` [33, 1024] bf16
    (row 0 = 1.0 for the bias term).

  Phase 2 (fused diag+cross): per chunk and r-half,
    psum  = D_c^T @ x_c      (start, upper-tri intra-chunk block)
    psum += M_c^T @ sall     (stop; M_c rows hold bias / w1*dv1^(t-e_c')
                              / dv0^(t-e_c') cross-chunk factors)
    then a single [128, 1024] PSUM->SBUF bf16 copy per chunk
    (alternating DVE/ACT) into a 2-chunk staging tile, stored as
    512 KB DMAs alternating the two HWDGE rings.

The host ships x^T pre-cast to bf16 in a chunk-tiled [128, 16*1024]
layout (so every load/store is per-partition contiguous), and
un-permutes / casts the bf16 result back to fp32.
"""

import sys

if "/opt/trn_rl_repo" not in sys.path:
    sys.path.insert(0, "/opt/trn_rl_repo")

import numpy as np
import ml_dtypes

import concourse.mybir as mybir
from concourse import bacc
from concourse.bass_utils import run_bass_kernel_spmd
from concourse.tile import TileContext

_P = 128
_B, _E, _S = 4, 2048, 2048
_NCORES = 8
_R = (_B * _E) // _NCORES  # 1024 rows (r) per core
_NCH = _S // _P  # 16 chunks of 128 along S
_NS = 1 + 2 * _NCH  # 33 sall rows (bias row + 2 per chunk)
_HALF = 512  # r per matmul (one PSUM bank, fp32)

_BF16 = mybir.dt.bfloat16
_F32 = mybir.dt.float32
_NPBF16 = ml_dtypes.bfloat16


def _build_host_mats(w0, w1, dv0, dv1, bias):
    """Build D [128, 16*128], G [128, 16*33], M [33, 16*128] (float64->bf16)."""
    w0 = w0.astype(np.float64)
    w1 = w1.astype(np.float64)
    bias = bias.astype(np.float64)
    s = np.arange(_P)[:, None]
    t = np.arange(_P)[None, :]
    mask = t >= s
    e = np.where(mask, t - s, 0).astype(np.float64)
    rev = np.arange(_P - 1, -1, -1).astype(np.float64)  # 127 - s

    D = np.zeros((_P, _NCH * _P), dtype=np.float64)
    G = np.zeros((_P, _NCH * _NS), dtype=np.float64)
    M = np.zeros((_NS, _NCH * _P), dtype=np.float64)
    for c in range(_NCH):
        base = c * _P
        blk = np.where(
            mask,
            w0[base : base + _P][:, None] * (dv0**e)
            + w1[base : base + _P][None, :] * (dv1**e),
            0.0,
        )
        D[:, c * _P : (c + 1) * _P] = blk
        G[:, c * _NS + 1 + 2 * c] = dv1**rev
        G[:, c * _NS + 2 + 2 * c] = w0[base : base + _P] * (dv0**rev)
        tg = base + np.arange(_P)
        M[0, c * _P : (c + 1) * _P] = bias[tg]
        for cp in range(c):
            e_cp = cp * _P + _P - 1
            M[1 + 2 * cp, c * _P : (c + 1) * _P] = w1[tg] * (dv1 ** (tg - e_cp))
            M[2 + 2 * cp, c * _P : (c + 1) * _P] = dv0 ** (tg - e_cp)
    return (
        D.astype(_NPBF16),
        G.astype(_NPBF16),
        M.astype(_NPBF16),
    )


def _build(with_bias):
    nc = bacc.Bacc(
        "TRN2",
        target_bir_lowering=False,
        debug=False,
        enable_asserts=False,
        num_devices=_NCORES,
    )
    xt = nc.dram_tensor("xt", [_P, _NCH * _R], _BF16, kind="ExternalInput").ap()
    Dd = nc.dram_tensor("Dd", [_P, _NCH * _P], _BF16, kind="ExternalInput").ap()
    Gd = nc.dram_tensor("Gd", [_P, _NCH * _NS], _BF16, kind="ExternalInput").ap()
    Md = nc.dram_tensor("Md", [_NS, _NCH * _P], _BF16, kind="ExternalInput").ap()
    outT = nc.dram_tensor("outT", [_P, _NCH * _R], _BF16, kind="ExternalOutput").ap()

    with TileContext(nc) as tc:
        with (
            tc.tile_pool(name="consts", bufs=1) as cpool,
            tc.tile_pool(name="xin", bufs=4) as xpool,
            tc.tile_pool(name="ot", bufs=3) as otpool,
            tc.tile_pool(name="ps", bufs=1, space="PSUM") as pspool,
            tc.tile_pool(name="po", bufs=3, space="PSUM") as popool,
        ):
            Gt = cpool.tile([_P, _NCH * _NS], _BF16)
            Mt = cpool.tile([_NS, _NCH * _P], _BF16)
            Dt = cpool.tile([_P, _NCH * _P], _BF16)
            sall = cpool.tile([_NS, _R], _BF16)
            # small constants off the HWDGE rings (SWDGE is idle otherwise)
            nc.gpsimd.dma_start(Gt[:], Gd[:])
            nc.gpsimd.dma_start(Mt[:], Md[:])
            nc.scalar.dma_start(Dt[:], Dd[:])

            # x: 4 blocks of 4 chunks, ~1 MB per DMA, alternating rings
            xb = []
            for b in range(4):
                xbt = xpool.tile([_P, 4 * _R], _BF16, tag="xb", name="xb")
                eng = nc.sync if b % 2 == 0 else nc.scalar
                eng.dma_start(xbt[:], xt[:, b * 4 * _R : (b + 1) * 4 * _R])
                xb.append(xbt)

            def xap(c, h):
                b, j = divmod(c, 4)
                lo = j * _R + h * _HALF
                return xb[b][:, lo : lo + _HALF]

            # ---- Phase 1: chunk reductions into standing PSUM banks ----
            psh = [
                pspool.tile([_NS, _HALF], _F32, tag="psA", name="psA"),
                pspool.tile([_NS, _HALF], _F32, tag="psB", name="psB"),
            ]
            for c in range(_NCH):
                for h in (0, 1):
                    nc.tensor.matmul(
                        psh[h][:],
                        Gt[:, c * _NS : (c + 1) * _NS],
                        xap(c, h),
                        start=(c == 0),
                        stop=(c == _NCH - 1),
                    )
            # engine ops need 32-aligned partition bases: copy all 33 rows
            # (row 0 is zero from G's zero column) and, only when the bias
            # row is live, overwrite row 0 with the 1.0 the bias row of M
            # multiplies.
            nc.vector.tensor_copy(sall[:, 0:_HALF], psh[0][:])
            nc.scalar.copy(sall[:, _HALF : 2 * _HALF], psh[1][:])
            if with_bias:
                nc.gpsimd.memset(sall[0:1, :], 1.0)

            # ---- Phase 2: fused diag + cross, 3-chunk diag lookahead ----
            po_of = {}

            def emit_diag(c):
                po = popool.tile([_P, 2 * _HALF], _F32, tag="po", name="po")
                for h in (0, 1):
                    nc.tensor.matmul(
                        po[:, h * _HALF : (h + 1) * _HALF],
                        Dt[:, c * _P : (c + 1) * _P],
                        xap(c, h),
                        start=True,
                        stop=False,
                    )
                po_of[c] = po

            for c in range(3):
                emit_diag(c)
            ot = None
            for c in range(_NCH):
                po = po_of.pop(c)
                for h in (0, 1):
                    nc.tensor.matmul(
                        po[:, h * _HALF : (h + 1) * _HALF],
                        Mt[:, c * _P : (c + 1) * _P],
                        sall[:, h * _HALF : (h + 1) * _HALF],
                        start=False,
                        stop=True,
                    )
                if c % 2 == 0:
                    ot = otpool.tile([_P, 2 * _R], _BF16, tag="ot", name="ot")
                dst = ot[:, (c % 2) * _R : (c % 2 + 1) * _R]
                if c % 2 == 0:
                    nc.vector.tensor_copy(dst, po[:])
                else:
                    nc.scalar.copy(dst, po[:])
                if c + 3 < _NCH:
                    emit_diag(c + 3)
                if c % 2 == 1:
                    eng = nc.sync if (c // 2) % 2 == 0 else nc.scalar
                    eng.dma_start(outT[:, (c - 1) * _R : (c + 1) * _R], ot[:])
    nc.compile()
    return nc


def _shard_x(x):
    """x [B, E, S] fp32 -> per-core chunk-tiled x^T [128, NCH*R] bf16."""
    xf = np.asarray(x, dtype=np.float32).reshape(_B * _E, _S)
    xT = np.ascontiguousarray(xf.T)  # [S, B*E]
    shards = []
    for c in range(_NCORES):
        xc = xT[:, c * _R : (c + 1) * _R]  # [S, R]
        xc = np.ascontiguousarray(xc).reshape(_NCH, _P, _R).transpose(1, 0, 2)
        shards.append(np.ascontiguousarray(xc.astype(_NPBF16)).reshape(_P, _NCH * _R))
    return shards


def _unshard_out(parts):
    """per-core [128, NCH*R] bf16 -> [B, E, S] fp32."""
    cols = []
    for p in parts:
        pc = p.reshape(_P, _NCH, _R).transpose(1, 0, 2).reshape(_S, _R)
        cols.append(pc)
    outT = np.concatenate(cols, axis=1)  # [S, B*E] bf16
    return np.ascontiguousarray(outT.T).astype(np.float32).reshape(_B, _E, _S)


def _run(x, weight, bias, decay_value, trace=False):
    w = np.asarray(weight, dtype=np.float32)
    b = np.asarray(bias, dtype=np.float32)
    dv = np.asarray(decay_value, dtype=np.float32)
    dv0 = float(np.clip(dv[0, 0], 0.9, 1.0))
    dv1 = float(np.clip(dv[1, 0], 0.9, 1.0))

    D, G, M = _build_host_mats(w[0], w[1], dv0, dv1, b)
    nc = _build(bool(np.any(b)))

    shards = _shard_x(x)
    in_maps = [
        {"xt": shards[c], "Dd": D, "Gd": G, "Md": M} for c in range(_NCORES)
    ]

    res = run_bass_kernel_spmd(nc, in_maps, core_ids=list(range(_NCORES)), trace=trace)
    full = _unshard_out([res.results[c]["outT"] for c in range(_NCORES)])
    return full, res


def kernel(x, weight, bias, decay_value):
    full, _ = _run(x, weight, bias, decay_value, trace=False)
    return full
